# revision 7
# baseline (speedup 1.0000x reference)
"""Multi-head self-attention (B=4, T=2048, D=768, H=12, d_k=64) on 8 trn2 cores.

Sharding: core c handles batch c//2 and head-group c%2 (6 heads = 3 pairs).
Each core computes its heads' attention plus its rows of the output
projection; the host sums the two partial projections per batch and adds bo.

Device dataflow (fp16 matmul operands, fp32 PSUM accumulation):
  - host passes q/k transposed (d-major) so no on-chip transposes are needed
  - per-head-pair projections / S^T / PV are packed into the 128x128 PE via
    partition-offset tile placement (two K=64 or M=64 matmuls run concurrently)
  - softmax: exp(s - 5) on ACT (shift keeps fp16 in range; ratios unchanged;
    1/sqrt(d_k) folded into Wq/bq on the host), denominators via all-ones
    stationary matmuls (result lands replicated across partitions),
    normalization deferred to after PV using a fast DVE reciprocal
  - P @ V uses raw v; Wv is applied after PV (linearity), bv added exactly
    because softmax rows sum to 1
"""

import os

import numpy as np

import concourse.bass as bass
import concourse.mybir as mybir
from concourse.bass import ds
from concourse.bass_utils import run_bass_kernel_spmd
from concourse.tile import TileContext
from concourse.vector_clock import ScopedClock, VectorClock

B, T, D = 4, 2048, 768
H, DK = 12, 64
HPC = 6          # heads per core
SLC = HPC * DK   # 384 feature columns per core
N_CORES = 8
ESHIFT = -5.0    # exp(s + ESHIFT): keeps exp/denexisting sums in fp16 range

F16 = mybir.dt.float16
F32 = mybir.dt.float32


def _patch_tile_drain():
    """Walrus CoreV3 CTRL lowering in this build rejects >1 sem wait on the
    TileContext-exit Drain. Split the waits across single-wait nops."""
    if getattr(TileContext, "_drain_patched", False):
        return

    def _drain_and_barrier(self, tick_clock, wait_clock):
        vc = tick_clock.global_clock
        for proc in range(len(vc)):
            t = vc[proc]
            if t > 0:
                nop_inst = self.nc.sync.nop(nofuse=True, hint="drain_wait_split")
                vec = [0] * len(vc)
                vec[proc] = t
                wait_clock.add_sem_waits(
                    nop_inst.ins, ScopedClock({None: VectorClock(vec)})
                )
        self.nc.sync.drain()
        self.nc.all_engine_barrier()
        assert self.sems is not None
        popped = self.nc._tile_sem_poison_stack.pop()
        assert popped is self._sem_poison
        self.nc.clear_and_free_semaphores(list(self.sems.allocated().values()))
        self.nc.all_engine_barrier()

    TileContext._drain_and_barrier = _drain_and_barrier

    # The same walrus build accepts at most ONE sem wait per instruction.
    # Tile's scheduler attaches 2-4. Peel all but the last wait onto
    # single-wait same-engine NoOps at instruction-commit time.
    orig_add = TileContext._add_instruction

    def _add_instruction(self, inst):
        si = getattr(inst, "sync_info", None)
        if si is not None and si.on_wait is not None and len(si.on_wait) > 1:
            waits = list(si.on_wait)
            for w in waits[:-1]:
                nop = mybir.InstNoOp(
                    name=self.nc.get_next_instruction_name(),
                    ins=[],
                    outs=[],
                    text_hint="wait_split",
                    bass_nofuse=True,
                )
                nop.engine = inst.engine
                nop.sync_info = mybir.SyncInfo(on_wait=[w], on_update=[])
                orig_add(self, nop)
            si.on_wait = waits[-1:]
        orig_add(self, inst)

    TileContext._add_instruction = _add_instruction
    TileContext._drain_patched = True


def _install_trace_hook():
    """Provide the antenv.axon_hooks NTFF profile hook this container lacks,
    and skip the bucket artifact upload. Only used when KERNEL_TRACE is set."""
    import sys
    import types

    import concourse.bass_utils as bass_utils

    bass_utils.upload_artifacts = lambda tmpdir: f"local://{tmpdir}"
    if "antenv.axon_hooks" not in sys.modules:
        mod = types.ModuleType("antenv.axon_hooks")
        mod._hook = None
        mod.set_axon_ntff_profile_hook = lambda h: setattr(mod, "_hook", h)
        mod.get_axon_ntff_profile_hook = lambda: mod._hook
        sys.modules["antenv.axon_hooks"] = mod
    from trn_agent_boot.trn_boot import _ntff_profile_via_ctypes

    sys.modules["antenv.axon_hooks"].set_axon_ntff_profile_hook(
        _ntff_profile_via_ctypes("/opt/axon/libaxon_pjrt.so")
    )


def _build_bass():
    _patch_tile_drain()
    nc = bass.Bass("TRN2", target_bir_lowering=False, debug=False)

    qT_d = nc.dram_tensor("qT", [SLC, T], F16, kind="ExternalInput")
    kT_d = nc.dram_tensor("kT", [SLC, T], F16, kind="ExternalInput")
    vN_d = nc.dram_tensor("vN", [T, SLC], F16, kind="ExternalInput")
    wq_d = nc.dram_tensor("wqT", [128, DK], F16, kind="ExternalInput")
    wk_d = nc.dram_tensor("wkT", [128, DK], F16, kind="ExternalInput")
    wv_d = nc.dram_tensor("wvT", [128, DK], F16, kind="ExternalInput")
    bq_d = nc.dram_tensor("bq2", [128, 1], F32, kind="ExternalInput")
    bk_d = nc.dram_tensor("bk2", [128, 1], F32, kind="ExternalInput")
    bv_d = nc.dram_tensor("bv2", [128, 1], F32, kind="ExternalInput")
    wo_d = nc.dram_tensor("woT", [SLC, D], F16, kind="ExternalInput")
    out_d = nc.dram_tensor("out", [T, D], F32, kind="ExternalOutput")

    NP = SLC // 128  # 3 head-pairs
    NG = T // 512    # 4 query groups
    NC = T // 128    # 16 key chunks

    with TileContext(nc) as tc:
        with (
            tc.tile_pool(name="consts", bufs=1) as cst,
            tc.tile_pool(name="inp", bufs=1) as inp,
        ):
            # --- constants / weights ---
            wq = cst.tile([128, DK], F16, tag="wq")
            wk = cst.tile([128, DK], F16, tag="wk")
            wv = cst.tile([128, DK], F16, tag="wv")
            nc.sync.dma_start(out=wq[:], in_=wq_d[:])
            nc.sync.dma_start(out=wk[:], in_=wk_d[:])
            nc.sync.dma_start(out=wv[:], in_=wv_d[:])
            bqs = cst.tile([128, 1], F32, tag="bqs")
            bks = cst.tile([128, 1], F32, tag="bks")
            bvs = cst.tile([128, 1], F32, tag="bvs")
            nc.sync.dma_start(out=bqs[:], in_=bq_d[:])
            nc.sync.dma_start(out=bks[:], in_=bk_d[:])
            nc.sync.dma_start(out=bvs[:], in_=bv_d[:])
            ones = cst.tile([128, DK], F16, tag="ones")
            nc.vector.memset(ones[:], 1.0)
            ebias = cst.tile([128, 1], F32, tag="ebias")
            nc.vector.memset(ebias[:], ESHIFT)

            # --- inputs ---
            qTr = []
            kTr = []
            for p in range(NP):
                tq = inp.tile([128, T], F16, tag=f"qTr{p}")
                nc.sync.dma_start(
                    out=tq[:], in_=qT_d.rearrange("(n p) t -> n p t", p=128)[p]
                )
                qTr.append(tq)
                tk = inp.tile([128, T], F16, tag=f"kTr{p}")
                nc.sync.dma_start(
                    out=tk[:], in_=kT_d.rearrange("(n p) t -> n p t", p=128)[p]
                )
                kTr.append(tk)
            vS = inp.tile([128, NC * SLC], F16, tag="vS")
            nc.sync.dma_start(
                out=vS[:].rearrange("p (n d) -> p n d", n=NC),
                in_=vN_d.rearrange("(n p) d -> p n d", p=128),
            )
            woS = []
            for p in range(NP):
                tw = inp.tile([128, D], F16, tag=f"woS{p}")
                nc.sync.dma_start(
                    out=tw[:], in_=wo_d.rearrange("(n p) o -> n p o", p=128)[p]
                )
                woS.append(tw)

            qhT = [
                inp.tile([128, T], F16, tag=f"qhT{p}", name=f"qhT{p}")
                for p in range(NP)
            ]
            khT = [
                inp.tile([128, T], F16, tag=f"khT{p}", name=f"khT{p}")
                for p in range(NP)
            ]
            XT = [
                inp.tile([128, T], F16, tag=f"XT{p}", name=f"XT{p}")
                for p in range(NP)
            ]

            # --- q/k projections (pair-packed: (0,0) and (64,64)) ---
            with tc.tile_pool(name="pjp", bufs=2, space="PSUM") as pjp:
                for p in range(NP):
                    for src, w, bias, dst in (
                        (qTr[p], wq, bqs, qhT[p]),
                        (kTr[p], wk, bks, khT[p]),
                    ):
                        pj = pjp.tile([128, T], F32, tag="pj")
                        for n in range(NG):
                            sl = ds(n * 512, 512)
                            nc.tensor.matmul(
                                pj[0:64, sl], w[0:64, :], src[0:64, sl],
                                start=True, stop=True,
                            )
                            nc.tensor.matmul(
                                pj[64:128, sl], w[64:128, :], src[64:128, sl],
                                start=True, stop=True,
                            )
                        nc.vector.tensor_scalar_add(dst[:], pj[:], bias[:])

            # --- attention ---
            with (
                tc.tile_pool(name="stp", bufs=2, space="PSUM") as stp,
                tc.tile_pool(name="dnp", bufs=1, space="PSUM") as dnp,
                tc.tile_pool(name="up", bufs=2, space="PSUM") as up,
                tc.tile_pool(name="otp", bufs=1, space="PSUM") as otp,
                tc.tile_pool(name="ptp", bufs=17) as ptp,
                tc.tile_pool(name="sbt", bufs=3) as sbt,
            ):
                for p in range(NP):
                    for g in range(NG):
                        gsl = ds(g * 512, 512)
                        pts = []
                        for c in range(NC):
                            csl = ds(c * 128, 128)
                            st = stp.tile([128, 1024], F32, tag="st")
                            nc.tensor.matmul(
                                st[:, 0:512], khT[p][0:64, csl], qhT[p][0:64, gsl],
                                start=True, stop=True,
                            )
                            nc.tensor.matmul(
                                st[:, 512:1024], khT[p][64:128, csl],
                                qhT[p][64:128, gsl],
                                start=True, stop=True,
                            )
                            pt = ptp.tile([128, 1024], F16, tag="pt")
                            nc.scalar.activation(
                                pt[:], st[:],
                                mybir.ActivationFunctionType.Exp,
                                bias=ebias[:], scale=1.0,
                            )
                            pts.append(pt)
                        dn = dnp.tile([128, 512], F32, tag="dn")
                        u = up.tile([128, 512], F32, tag="u")
                        for c in range(NC):
                            pt = pts[c]
                            first, last = c == 0, c == NC - 1
                            voff = c * SLC + p * 128
                            nc.tensor.matmul(
                                dn[0:64, :], ones[:, :], pt[:, 0:512],
                                start=first, stop=last, skip_group_check=True,
                            )
                            nc.tensor.matmul(
                                dn[64:128, :], ones[:, :], pt[:, 512:1024],
                                start=first, stop=last, skip_group_check=True,
                            )
                            nc.tensor.matmul(
                                u[0:64, :], vS[:, ds(voff, 64)], pt[:, 0:512],
                                start=first, stop=last, skip_group_check=True,
                            )
                            nc.tensor.matmul(
                                u[64:128, :], vS[:, ds(voff + 64, 64)],
                                pt[:, 512:1024],
                                start=first, stop=last, skip_group_check=True,
                            )
                        # normalize + Wv + bv
                        rcp = sbt.tile([128, 512], F32, tag="rcp")
                        nc.vector.reciprocal(rcp[:], dn[:])
                        un = sbt.tile([128, 512], F16, tag="un")
                        nc.vector.tensor_tensor(
                            un[:], u[:], rcp[:], op=mybir.AluOpType.mult
                        )
                        ot = otp.tile([128, 512], F32, tag="ot")
                        nc.tensor.matmul(
                            ot[0:64, :], wv[0:64, :], un[0:64, :],
                            start=True, stop=True,
                        )
                        nc.tensor.matmul(
                            ot[64:128, :], wv[64:128, :], un[64:128, :],
                            start=True, stop=True,
                        )
                        nc.vector.tensor_scalar_add(XT[p][:, gsl], ot[:], bvs[:])

            # --- output projection ---
            with (
                tc.tile_pool(name="pop", bufs=2, space="PSUM") as pop,
                tc.tile_pool(name="outp", bufs=3) as outp,
            ):
                for qb in range(NC):
                    qsl = ds(qb * 128, 128)
                    po = pop.tile([128, D], F32, tag="po")
                    for p in range(NP):
                        first, last = p == 0, p == NP - 1
                        nc.tensor.matmul(
                            po[:, 0:512], XT[p][:, qsl], woS[p][:, 0:512],
                            start=first, stop=last,
                        )
                        nc.tensor.matmul(
                            po[:, 512:768], XT[p][:, qsl], woS[p][:, 512:768],
                            start=first, stop=last,
                        )
                    ou = outp.tile([128, D], F32, tag="ou")
                    nc.vector.tensor_copy(ou[:], po[:])
                    nc.sync.dma_start(
                        out=out_d.rearrange("(n p) o -> n p o", p=128)[qb],
                        in_=ou[:],
                    )

    return nc


def kernel(q, k, v, Wq, bq, Wk, bk, Wv, bv, Wo, bo):
    q = np.asarray(q, dtype=np.float32)
    k = np.asarray(k, dtype=np.float32)
    v = np.asarray(v, dtype=np.float32)
    Wq = np.asarray(Wq, dtype=np.float32)
    bq = np.asarray(bq, dtype=np.float32)
    Wk = np.asarray(Wk, dtype=np.float32)
    bk = np.asarray(bk, dtype=np.float32)
    Wv = np.asarray(Wv, dtype=np.float32)
    bv = np.asarray(bv, dtype=np.float32)
    Wo = np.asarray(Wo, dtype=np.float32)
    bo = np.asarray(bo, dtype=np.float32)

    s = 1.0 / np.sqrt(DK)
    wqT2 = np.concatenate([Wq.T * s, Wq.T * s], axis=0).astype(np.float16)
    wkT2 = np.concatenate([Wk.T, Wk.T], axis=0).astype(np.float16)
    wvT2 = np.concatenate([Wv.T, Wv.T], axis=0).astype(np.float16)
    bq2 = np.concatenate([bq * s, bq * s])[:, None].astype(np.float32)
    bk2 = np.concatenate([bk, bk])[:, None].astype(np.float32)
    bv2 = np.concatenate([bv, bv])[:, None].astype(np.float32)

    in_maps = []
    for c in range(N_CORES):
        b, hg = c // 2, c % 2
        cols = slice(hg * SLC, (hg + 1) * SLC)
        in_maps.append(
            {
                "qT": np.ascontiguousarray(q[b][:, cols].T).astype(np.float16),
                "kT": np.ascontiguousarray(k[b][:, cols].T).astype(np.float16),
                "vN": np.ascontiguousarray(v[b][:, cols]).astype(np.float16),
                "wqT": wqT2,
                "wkT": wkT2,
                "wvT": wvT2,
                "bq2": bq2,
                "bk2": bk2,
                "bv2": bv2,
                "woT": np.ascontiguousarray(Wo[:, cols].T).astype(np.float16),
            }
        )

    nc = _build_bass()
    trace = bool(os.environ.get("KERNEL_TRACE"))
    if trace:
        _install_trace_hook()
    tmpdir = os.environ.get("KERNEL_TRACE_DIR") or None
    res = run_bass_kernel_spmd(
        nc, in_maps, list(range(N_CORES)), trace=trace, tmpdir=tmpdir
    )
    if trace:
        print("KERNEL exec_time_ns:", res.exec_time_ns)
        kernel.last_results = res

    out = np.zeros((B, T, D), dtype=np.float32)
    for b in range(B):
        out[b] = res.results[2 * b]["out"] + res.results[2 * b + 1]["out"] + bo[None, :]
    return out


# revision 11
# speedup vs baseline: 1.0380x; 1.0380x over previous
"""Multi-head self-attention (B=4, T=2048, D=768, H=12, d_k=64) on 8 trn2 cores.

Sharding: core c handles batch c//2 and head-group c%2 (6 heads = 3 pairs).
Each core computes its heads' attention plus its rows of the output
projection; the host sums the two partial projections per batch and adds bo.

Device dataflow (fp16 matmul operands, fp32 PSUM accumulation):
  - host passes q/k transposed (d-major) so no on-chip transposes are needed
  - per-head-pair projections / S^T / PV are packed into the 128x128 PE via
    partition-offset tile placement (two K=64 or M=64 matmuls run concurrently)
  - softmax: exp(s - 5) on ACT (shift keeps fp16 in range; ratios unchanged;
    1/sqrt(d_k) folded into Wq/bq on the host), denominators via all-ones
    stationary matmuls (result lands replicated across partitions),
    normalization deferred to after PV using a fast DVE reciprocal
  - P @ V uses raw v; Wv is applied after PV (linearity), bv added exactly
    because softmax rows sum to 1
"""

import os

import numpy as np

import concourse.bass as bass
import concourse.mybir as mybir
from concourse.bass import ds
from concourse.bass_utils import run_bass_kernel_spmd
from concourse.tile import TileContext
from concourse.vector_clock import ScopedClock, VectorClock

B, T, D = 4, 2048, 768
H, DK = 12, 64
HPC = 6          # heads per core
SLC = HPC * DK   # 384 feature columns per core
N_CORES = 8
ESHIFT = -5.0    # exp(s + ESHIFT): keeps exp/denexisting sums in fp16 range

F16 = mybir.dt.float16
F32 = mybir.dt.float32


def _patch_tile_drain():
    """Walrus CoreV3 CTRL lowering in this build rejects >1 sem wait on the
    TileContext-exit Drain. Split the waits across single-wait nops."""
    if getattr(TileContext, "_drain_patched", False):
        return

    def _drain_and_barrier(self, tick_clock, wait_clock):
        vc = tick_clock.global_clock
        for proc in range(len(vc)):
            t = vc[proc]
            if t > 0:
                nop_inst = self.nc.sync.nop(nofuse=True, hint="drain_wait_split")
                vec = [0] * len(vc)
                vec[proc] = t
                wait_clock.add_sem_waits(
                    nop_inst.ins, ScopedClock({None: VectorClock(vec)})
                )
        self.nc.sync.drain()
        self.nc.all_engine_barrier()
        assert self.sems is not None
        popped = self.nc._tile_sem_poison_stack.pop()
        assert popped is self._sem_poison
        self.nc.clear_and_free_semaphores(list(self.sems.allocated().values()))
        self.nc.all_engine_barrier()

    TileContext._drain_and_barrier = _drain_and_barrier

    # The same walrus build accepts at most ONE sem wait per instruction.
    # Tile's scheduler attaches 2-4. Peel all but the last wait onto
    # single-wait same-engine NoOps at instruction-commit time.
    orig_add = TileContext._add_instruction

    def _add_instruction(self, inst):
        si = getattr(inst, "sync_info", None)
        if si is not None and si.on_wait is not None and len(si.on_wait) > 1:
            waits = list(si.on_wait)
            for w in waits[:-1]:
                nop = mybir.InstNoOp(
                    name=self.nc.get_next_instruction_name(),
                    ins=[],
                    outs=[],
                    text_hint="wait_split",
                    bass_nofuse=True,
                )
                nop.engine = inst.engine
                nop.sync_info = mybir.SyncInfo(on_wait=[w], on_update=[])
                orig_add(self, nop)
            si.on_wait = waits[-1:]
        orig_add(self, inst)

    TileContext._add_instruction = _add_instruction
    TileContext._drain_patched = True


def _install_trace_hook():
    """Provide the antenv.axon_hooks NTFF profile hook this container lacks,
    and skip the bucket artifact upload. Only used when KERNEL_TRACE is set."""
    import sys
    import types

    import concourse.bass_utils as bass_utils

    bass_utils.upload_artifacts = lambda tmpdir: f"local://{tmpdir}"
    if "antenv.axon_hooks" not in sys.modules:
        mod = types.ModuleType("antenv.axon_hooks")
        mod._hook = None
        mod.set_axon_ntff_profile_hook = lambda h: setattr(mod, "_hook", h)
        mod.get_axon_ntff_profile_hook = lambda: mod._hook
        sys.modules["antenv.axon_hooks"] = mod
    from trn_agent_boot.trn_boot import _ntff_profile_via_ctypes

    sys.modules["antenv.axon_hooks"].set_axon_ntff_profile_hook(
        _ntff_profile_via_ctypes("/opt/axon/libaxon_pjrt.so")
    )


def _build_bass():
    _patch_tile_drain()
    nc = bass.Bass("TRN2", target_bir_lowering=False, debug=False)

    qT_d = nc.dram_tensor("qT", [SLC, T], F16, kind="ExternalInput")
    kT_d = nc.dram_tensor("kT", [SLC, T], F16, kind="ExternalInput")
    vN_d = nc.dram_tensor("vN", [T, SLC], F16, kind="ExternalInput")
    wq_d = nc.dram_tensor("wqT", [128, DK], F16, kind="ExternalInput")
    wk_d = nc.dram_tensor("wkT", [128, DK], F16, kind="ExternalInput")
    wv_d = nc.dram_tensor("wvT", [128, DK], F16, kind="ExternalInput")
    bq_d = nc.dram_tensor("bq2", [128, 1], F32, kind="ExternalInput")
    bk_d = nc.dram_tensor("bk2", [128, 1], F32, kind="ExternalInput")
    bv_d = nc.dram_tensor("bv2", [128, 1], F32, kind="ExternalInput")
    wo_d = nc.dram_tensor("woT", [SLC, D], F16, kind="ExternalInput")
    out_d = nc.dram_tensor("out", [T, D], F32, kind="ExternalOutput")

    NP = SLC // 128  # 3 head-pairs
    NG = T // 512    # 4 query groups
    NC = T // 128    # 16 key chunks

    with TileContext(nc) as tc:
        with (
            tc.tile_pool(name="consts", bufs=1) as cst,
            tc.tile_pool(name="inp", bufs=1) as inp,
        ):
            # --- constants / weights ---
            wq = cst.tile([128, DK], F16, tag="wq")
            wk = cst.tile([128, DK], F16, tag="wk")
            wv = cst.tile([128, DK], F16, tag="wv")
            nc.sync.dma_start(out=wq[:], in_=wq_d[:])
            nc.sync.dma_start(out=wk[:], in_=wk_d[:])
            nc.sync.dma_start(out=wv[:], in_=wv_d[:])
            bqs = cst.tile([128, 1], F32, tag="bqs")
            bks = cst.tile([128, 1], F32, tag="bks")
            bvs = cst.tile([128, 1], F32, tag="bvs")
            nc.sync.dma_start(out=bqs[:], in_=bq_d[:])
            nc.sync.dma_start(out=bks[:], in_=bk_d[:])
            nc.sync.dma_start(out=bvs[:], in_=bv_d[:])
            ones = cst.tile([128, DK], F16, tag="ones")
            nc.vector.memset(ones[:], 1.0)
            ebias = cst.tile([128, 1], F32, tag="ebias")
            nc.vector.memset(ebias[:], ESHIFT)

            # --- inputs ---
            # q/k loads are interleaved with the per-pair pipeline below; v
            # and Wo are queued right after pair 0's q/k so they land before
            # first use.
            qTr = [
                inp.tile([128, T], F16, tag=f"qTr{p}", name=f"qTr{p}")
                for p in range(NP)
            ]
            kTr = [
                inp.tile([128, T], F16, tag=f"kTr{p}", name=f"kTr{p}")
                for p in range(NP)
            ]
            nc.sync.dma_start(
                out=qTr[0][:], in_=qT_d.rearrange("(n p) t -> n p t", p=128)[0]
            )
            nc.sync.dma_start(
                out=kTr[0][:], in_=kT_d.rearrange("(n p) t -> n p t", p=128)[0]
            )
            vS = inp.tile([128, NC * SLC], F16, tag="vS")
            nc.sync.dma_start(
                out=vS[:].rearrange("p (n d) -> p n d", n=NC),
                in_=vN_d.rearrange("(n p) d -> p n d", p=128),
            )
            for p in range(1, NP):
                nc.sync.dma_start(
                    out=qTr[p][:], in_=qT_d.rearrange("(n p) t -> n p t", p=128)[p]
                )
                nc.sync.dma_start(
                    out=kTr[p][:], in_=kT_d.rearrange("(n p) t -> n p t", p=128)[p]
                )
            woS = []
            for p in range(NP):
                tw = inp.tile([128, D], F16, tag=f"woS{p}", name=f"woS{p}")
                nc.sync.dma_start(
                    out=tw[:], in_=wo_d.rearrange("(n p) o -> n p o", p=128)[p]
                )
                woS.append(tw)

            qhT = [
                inp.tile([128, T], F16, tag=f"qhT{p}", name=f"qhT{p}")
                for p in range(NP)
            ]
            khT = [
                inp.tile([128, T], F16, tag=f"khT{p}", name=f"khT{p}")
                for p in range(NP)
            ]
            XT = [
                inp.tile([128, T], F16, tag=f"XT{p}", name=f"XT{p}")
                for p in range(NP)
            ]

            # --- attention (projections fused per pair; Wv stage deferred so
            # the PE never waits on the DVE reciprocal chain) ---
            with (
                tc.tile_pool(name="ptp", bufs=17) as ptp,
                tc.tile_pool(name="sbt", bufs=13) as sbt,
            ):
                uns = {}
                from contextlib import ExitStack

                attn_ctx = ExitStack()
                stp = attn_ctx.enter_context(
                    tc.tile_pool(name="stp", bufs=2, space="PSUM")
                )
                dnp = attn_ctx.enter_context(
                    tc.tile_pool(name="dnp", bufs=1, space="PSUM")
                )
                up = attn_ctx.enter_context(
                    tc.tile_pool(name="up", bufs=2, space="PSUM")
                )
                for p in range(NP):
                    # q/k projections for this pair (pair-packed (0,0)/(64,64));
                    # psum comes from the "st" slots
                    for src, w, bias, dst in (
                        (qTr[p], wq, bqs, qhT[p]),
                        (kTr[p], wk, bks, khT[p]),
                    ):
                        for h in range(2):
                            pjt = stp.tile([128, 1024], F32, tag="st", name="pj")
                            for n in range(2):
                                sl = ds(h * 1024 + n * 512, 512)
                                psl = ds(n * 512, 512)
                                nc.tensor.matmul(
                                    pjt[0:64, psl], w[0:64, :], src[0:64, sl],
                                    start=True, stop=True,
                                )
                                nc.tensor.matmul(
                                    pjt[64:128, psl], w[64:128, :], src[64:128, sl],
                                    start=True, stop=True,
                                )
                            nc.vector.tensor_scalar_add(
                                dst[:, ds(h * 1024, 1024)], pjt[:], bias[:]
                            )
                    for g in range(NG):
                        gsl = ds(g * 512, 512)
                        pts = []
                        for c in range(NC):
                            csl = ds(c * 128, 128)
                            st = stp.tile([128, 1024], F32, tag="st")
                            nc.tensor.matmul(
                                st[:, 0:512], khT[p][0:64, csl], qhT[p][0:64, gsl],
                                start=True, stop=True,
                            )
                            nc.tensor.matmul(
                                st[:, 512:1024], khT[p][64:128, csl],
                                qhT[p][64:128, gsl],
                                start=True, stop=True,
                            )
                            pt = ptp.tile([128, 1024], F16, tag="pt")
                            nc.scalar.activation(
                                pt[:], st[:],
                                mybir.ActivationFunctionType.Exp,
                                bias=ebias[:], scale=1.0,
                            )
                            pts.append(pt)
                        dn = dnp.tile([128, 512], F32, tag="dn")
                        u = up.tile([128, 512], F32, tag="u")
                        for c in range(NC):
                            pt = pts[c]
                            first, last = c == 0, c == NC - 1
                            voff = c * SLC + p * 128
                            nc.tensor.matmul(
                                dn[0:64, :], ones[:, :], pt[:, 0:512],
                                start=first, stop=last, skip_group_check=True,
                            )
                            nc.tensor.matmul(
                                dn[64:128, :], ones[:, :], pt[:, 512:1024],
                                start=first, stop=last, skip_group_check=True,
                            )
                            nc.tensor.matmul(
                                u[0:64, :], vS[:, ds(voff, 64)], pt[:, 0:512],
                                start=first, stop=last, skip_group_check=True,
                            )
                            nc.tensor.matmul(
                                u[64:128, :], vS[:, ds(voff + 64, 64)],
                                pt[:, 512:1024],
                                start=first, stop=last, skip_group_check=True,
                            )
                        # normalize on DVE (PE-independent); Wv deferred
                        rcp = sbt.tile([128, 512], F32, tag="rcp", bufs=2)
                        nc.vector.reciprocal(rcp[:], dn[:])
                        un = sbt.tile([128, 512], F16, tag="un", bufs=13)
                        nc.vector.tensor_tensor(
                            un[:], u[:], rcp[:], op=mybir.AluOpType.mult
                        )
                        uns[(p, g)] = un

                # --- deferred Wv projection + bv for all pairs ---
                attn_ctx.close()
                with tc.tile_pool(name="otp", bufs=2, space="PSUM") as otp:
                    for p in range(NP):
                        for g in range(NG):
                            un = uns[(p, g)]
                            ot = otp.tile([128, 512], F32, tag="ot")
                            nc.tensor.matmul(
                                ot[0:64, :], wv[0:64, :], un[0:64, :],
                                start=True, stop=True,
                            )
                            nc.tensor.matmul(
                                ot[64:128, :], wv[64:128, :], un[64:128, :],
                                start=True, stop=True,
                            )
                            nc.vector.tensor_scalar_add(
                                XT[p][:, ds(g * 512, 512)], ot[:], bvs[:]
                            )

            # --- output projection ---
            with (
                tc.tile_pool(name="pop", bufs=2, space="PSUM") as pop,
                tc.tile_pool(name="outp", bufs=3) as outp,
            ):
                for qb in range(NC):
                    qsl = ds(qb * 128, 128)
                    po = pop.tile([128, D], F32, tag="po")
                    for p in range(NP):
                        first, last = p == 0, p == NP - 1
                        nc.tensor.matmul(
                            po[:, 0:512], XT[p][:, qsl], woS[p][:, 0:512],
                            start=first, stop=last,
                        )
                        nc.tensor.matmul(
                            po[:, 512:768], XT[p][:, qsl], woS[p][:, 512:768],
                            start=first, stop=last,
                        )
                    ou = outp.tile([128, D], F32, tag="ou")
                    nc.vector.tensor_copy(ou[:], po[:])
                    nc.sync.dma_start(
                        out=out_d.rearrange("(n p) o -> n p o", p=128)[qb],
                        in_=ou[:],
                    )

    return nc


def kernel(q, k, v, Wq, bq, Wk, bk, Wv, bv, Wo, bo):
    q = np.asarray(q, dtype=np.float32)
    k = np.asarray(k, dtype=np.float32)
    v = np.asarray(v, dtype=np.float32)
    Wq = np.asarray(Wq, dtype=np.float32)
    bq = np.asarray(bq, dtype=np.float32)
    Wk = np.asarray(Wk, dtype=np.float32)
    bk = np.asarray(bk, dtype=np.float32)
    Wv = np.asarray(Wv, dtype=np.float32)
    bv = np.asarray(bv, dtype=np.float32)
    Wo = np.asarray(Wo, dtype=np.float32)
    bo = np.asarray(bo, dtype=np.float32)

    s = 1.0 / np.sqrt(DK)
    wqT2 = np.concatenate([Wq.T * s, Wq.T * s], axis=0).astype(np.float16)
    wkT2 = np.concatenate([Wk.T, Wk.T], axis=0).astype(np.float16)
    wvT2 = np.concatenate([Wv.T, Wv.T], axis=0).astype(np.float16)
    bq2 = np.concatenate([bq * s, bq * s])[:, None].astype(np.float32)
    bk2 = np.concatenate([bk, bk])[:, None].astype(np.float32)
    bv2 = np.concatenate([bv, bv])[:, None].astype(np.float32)

    in_maps = []
    for c in range(N_CORES):
        b, hg = c // 2, c % 2
        cols = slice(hg * SLC, (hg + 1) * SLC)
        in_maps.append(
            {
                "qT": np.ascontiguousarray(q[b][:, cols].T).astype(np.float16),
                "kT": np.ascontiguousarray(k[b][:, cols].T).astype(np.float16),
                "vN": np.ascontiguousarray(v[b][:, cols]).astype(np.float16),
                "wqT": wqT2,
                "wkT": wkT2,
                "wvT": wvT2,
                "bq2": bq2,
                "bk2": bk2,
                "bv2": bv2,
                "woT": np.ascontiguousarray(Wo[:, cols].T).astype(np.float16),
            }
        )

    nc = _build_bass()
    trace = bool(os.environ.get("KERNEL_TRACE"))
    if trace:
        _install_trace_hook()
    tmpdir = os.environ.get("KERNEL_TRACE_DIR") or None
    res = run_bass_kernel_spmd(
        nc, in_maps, list(range(N_CORES)), trace=trace, tmpdir=tmpdir
    )
    if trace:
        print("KERNEL exec_time_ns:", res.exec_time_ns)
        kernel.last_results = res

    out = np.zeros((B, T, D), dtype=np.float32)
    for b in range(B):
        out[b] = res.results[2 * b]["out"] + res.results[2 * b + 1]["out"] + bo[None, :]
    return out


# revision 12
# speedup vs baseline: 1.0417x; 1.0037x over previous
"""Multi-head self-attention (B=4, T=2048, D=768, H=12, d_k=64) on 8 trn2 cores.

Sharding: core c handles batch c//2 and head-group c%2 (6 heads = 3 pairs).
Each core computes its heads' attention plus its rows of the output
projection; the host sums the two partial projections per batch and adds bo.

Device dataflow (fp16 matmul operands, fp32 PSUM accumulation):
  - host passes q/k transposed (d-major) so no on-chip transposes are needed
  - per-head-pair projections / S^T / PV are packed into the 128x128 PE via
    partition-offset tile placement (two K=64 or M=64 matmuls run concurrently)
  - softmax: exp(s - 5) on ACT (shift keeps fp16 in range; ratios unchanged;
    1/sqrt(d_k) folded into Wq/bq on the host), denominators via all-ones
    stationary matmuls (result lands replicated across partitions),
    normalization deferred to after PV using a fast DVE reciprocal
  - P @ V uses raw v; Wv is applied after PV (linearity), bv added exactly
    because softmax rows sum to 1
"""

import os

import ml_dtypes
import numpy as np

import concourse.bass as bass
import concourse.mybir as mybir
from concourse.bass import ds
from concourse.bass_utils import run_bass_kernel_spmd
from concourse.tile import TileContext
from concourse.vector_clock import ScopedClock, VectorClock

B, T, D = 4, 2048, 768
H, DK = 12, 64
HPC = 6          # heads per core
SLC = HPC * DK   # 384 feature columns per core
N_CORES = 8
ESHIFT = -5.0    # exp(s + ESHIFT): keeps exp/denexisting sums in fp16 range

F16 = mybir.dt.float16
BF16 = mybir.dt.bfloat16
F32 = mybir.dt.float32


def _patch_tile_drain():
    """Walrus CoreV3 CTRL lowering in this build rejects >1 sem wait on the
    TileContext-exit Drain. Split the waits across single-wait nops."""
    if getattr(TileContext, "_drain_patched", False):
        return

    def _drain_and_barrier(self, tick_clock, wait_clock):
        vc = tick_clock.global_clock
        for proc in range(len(vc)):
            t = vc[proc]
            if t > 0:
                nop_inst = self.nc.sync.nop(nofuse=True, hint="drain_wait_split")
                vec = [0] * len(vc)
                vec[proc] = t
                wait_clock.add_sem_waits(
                    nop_inst.ins, ScopedClock({None: VectorClock(vec)})
                )
        self.nc.sync.drain()
        self.nc.all_engine_barrier()
        assert self.sems is not None
        popped = self.nc._tile_sem_poison_stack.pop()
        assert popped is self._sem_poison
        self.nc.clear_and_free_semaphores(list(self.sems.allocated().values()))
        self.nc.all_engine_barrier()

    TileContext._drain_and_barrier = _drain_and_barrier

    # The same walrus build accepts at most ONE sem wait per instruction.
    # Tile's scheduler attaches 2-4. Peel all but the last wait onto
    # single-wait same-engine NoOps at instruction-commit time.
    orig_add = TileContext._add_instruction

    def _add_instruction(self, inst):
        si = getattr(inst, "sync_info", None)
        if si is not None and si.on_wait is not None and len(si.on_wait) > 1:
            waits = list(si.on_wait)
            for w in waits[:-1]:
                nop = mybir.InstNoOp(
                    name=self.nc.get_next_instruction_name(),
                    ins=[],
                    outs=[],
                    text_hint="wait_split",
                    bass_nofuse=True,
                )
                nop.engine = inst.engine
                nop.sync_info = mybir.SyncInfo(on_wait=[w], on_update=[])
                orig_add(self, nop)
            si.on_wait = waits[-1:]
        orig_add(self, inst)

    TileContext._add_instruction = _add_instruction
    TileContext._drain_patched = True


def _install_trace_hook():
    """Provide the antenv.axon_hooks NTFF profile hook this container lacks,
    and skip the bucket artifact upload. Only used when KERNEL_TRACE is set."""
    import sys
    import types

    import concourse.bass_utils as bass_utils

    bass_utils.upload_artifacts = lambda tmpdir: f"local://{tmpdir}"
    if "antenv.axon_hooks" not in sys.modules:
        mod = types.ModuleType("antenv.axon_hooks")
        mod._hook = None
        mod.set_axon_ntff_profile_hook = lambda h: setattr(mod, "_hook", h)
        mod.get_axon_ntff_profile_hook = lambda: mod._hook
        sys.modules["antenv.axon_hooks"] = mod
    from trn_agent_boot.trn_boot import _ntff_profile_via_ctypes

    sys.modules["antenv.axon_hooks"].set_axon_ntff_profile_hook(
        _ntff_profile_via_ctypes("/opt/axon/libaxon_pjrt.so")
    )


def _build_bass():
    _patch_tile_drain()
    nc = bass.Bass("TRN2", target_bir_lowering=False, debug=False)

    qT_d = nc.dram_tensor("qT", [SLC, T], F16, kind="ExternalInput")
    kT_d = nc.dram_tensor("kT", [SLC, T], F16, kind="ExternalInput")
    vN_d = nc.dram_tensor("vN", [T, SLC], BF16, kind="ExternalInput")
    wq_d = nc.dram_tensor("wqT", [128, DK], F16, kind="ExternalInput")
    wk_d = nc.dram_tensor("wkT", [128, DK], F16, kind="ExternalInput")
    wv_d = nc.dram_tensor("wvT", [128, DK], BF16, kind="ExternalInput")
    bq_d = nc.dram_tensor("bq2", [128, 1], F32, kind="ExternalInput")
    bk_d = nc.dram_tensor("bk2", [128, 1], F32, kind="ExternalInput")
    bv_d = nc.dram_tensor("bv2", [128, 1], F32, kind="ExternalInput")
    wo_d = nc.dram_tensor("woT", [SLC, D], BF16, kind="ExternalInput")
    out_d = nc.dram_tensor("out", [T, D], F32, kind="ExternalOutput")

    NP = SLC // 128  # 3 head-pairs
    NG = T // 512    # 4 query groups
    NC = T // 128    # 16 key chunks

    with TileContext(nc) as tc:
        with (
            tc.tile_pool(name="consts", bufs=1) as cst,
            tc.tile_pool(name="inp", bufs=1) as inp,
        ):
            # --- constants / weights ---
            wq = cst.tile([128, DK], F16, tag="wq")
            wk = cst.tile([128, DK], F16, tag="wk")
            wv = cst.tile([128, DK], BF16, tag="wv")
            nc.sync.dma_start(out=wq[:], in_=wq_d[:])
            nc.sync.dma_start(out=wk[:], in_=wk_d[:])
            nc.sync.dma_start(out=wv[:], in_=wv_d[:])
            bqs = cst.tile([128, 1], F32, tag="bqs")
            bks = cst.tile([128, 1], F32, tag="bks")
            bvs = cst.tile([128, 1], F32, tag="bvs")
            nc.sync.dma_start(out=bqs[:], in_=bq_d[:])
            nc.sync.dma_start(out=bks[:], in_=bk_d[:])
            nc.sync.dma_start(out=bvs[:], in_=bv_d[:])
            ones = cst.tile([128, DK], BF16, tag="ones")
            nc.vector.memset(ones[:], 1.0)
            ebias = cst.tile([128, 1], F32, tag="ebias")
            nc.vector.memset(ebias[:], ESHIFT)

            # --- inputs ---
            # q/k loads are interleaved with the per-pair pipeline below; v
            # and Wo are queued right after pair 0's q/k so they land before
            # first use.
            qTr = [
                inp.tile([128, T], F16, tag=f"qTr{p}", name=f"qTr{p}")
                for p in range(NP)
            ]
            kTr = [
                inp.tile([128, T], F16, tag=f"kTr{p}", name=f"kTr{p}")
                for p in range(NP)
            ]
            nc.sync.dma_start(
                out=qTr[0][:], in_=qT_d.rearrange("(n p) t -> n p t", p=128)[0]
            )
            nc.sync.dma_start(
                out=kTr[0][:], in_=kT_d.rearrange("(n p) t -> n p t", p=128)[0]
            )
            vS = inp.tile([128, NC * SLC], BF16, tag="vS")
            nc.sync.dma_start(
                out=vS[:].rearrange("p (n d) -> p n d", n=NC),
                in_=vN_d.rearrange("(n p) d -> p n d", p=128),
            )
            for p in range(1, NP):
                nc.sync.dma_start(
                    out=qTr[p][:], in_=qT_d.rearrange("(n p) t -> n p t", p=128)[p]
                )
                nc.sync.dma_start(
                    out=kTr[p][:], in_=kT_d.rearrange("(n p) t -> n p t", p=128)[p]
                )
            woS = []
            for p in range(NP):
                tw = inp.tile([128, D], BF16, tag=f"woS{p}", name=f"woS{p}")
                nc.sync.dma_start(
                    out=tw[:], in_=wo_d.rearrange("(n p) o -> n p o", p=128)[p]
                )
                woS.append(tw)

            qhT = [
                inp.tile([128, T], F16, tag=f"qhT{p}", name=f"qhT{p}")
                for p in range(NP)
            ]
            khT = [
                inp.tile([128, T], F16, tag=f"khT{p}", name=f"khT{p}")
                for p in range(NP)
            ]
            XT = [
                inp.tile([128, T], BF16, tag=f"XT{p}", name=f"XT{p}")
                for p in range(NP)
            ]

            # --- attention (projections fused per pair; Wv stage deferred so
            # the PE never waits on the DVE reciprocal chain) ---
            with (
                tc.tile_pool(name="ptp", bufs=17) as ptp,
                tc.tile_pool(name="sbt", bufs=13) as sbt,
            ):
                uns = {}
                from contextlib import ExitStack

                attn_ctx = ExitStack()
                stp = attn_ctx.enter_context(
                    tc.tile_pool(name="stp", bufs=2, space="PSUM")
                )
                dnp = attn_ctx.enter_context(
                    tc.tile_pool(name="dnp", bufs=1, space="PSUM")
                )
                up = attn_ctx.enter_context(
                    tc.tile_pool(name="up", bufs=2, space="PSUM")
                )
                for p in range(NP):
                    # q/k projections for this pair (pair-packed (0,0)/(64,64));
                    # psum comes from the "st" slots
                    for src, w, bias, dst in (
                        (qTr[p], wq, bqs, qhT[p]),
                        (kTr[p], wk, bks, khT[p]),
                    ):
                        for h in range(2):
                            pjt = stp.tile([128, 1024], F32, tag="st", name="pj")
                            for n in range(2):
                                sl = ds(h * 1024 + n * 512, 512)
                                psl = ds(n * 512, 512)
                                nc.tensor.matmul(
                                    pjt[0:64, psl], w[0:64, :], src[0:64, sl],
                                    start=True, stop=True,
                                )
                                nc.tensor.matmul(
                                    pjt[64:128, psl], w[64:128, :], src[64:128, sl],
                                    start=True, stop=True,
                                )
                            nc.vector.tensor_scalar_add(
                                dst[:, ds(h * 1024, 1024)], pjt[:], bias[:]
                            )
                    for g in range(NG):
                        gsl = ds(g * 512, 512)
                        pts = []
                        for c in range(NC):
                            csl = ds(c * 128, 128)
                            st = stp.tile([128, 1024], F32, tag="st")
                            nc.tensor.matmul(
                                st[:, 0:512], khT[p][0:64, csl], qhT[p][0:64, gsl],
                                start=True, stop=True,
                            )
                            nc.tensor.matmul(
                                st[:, 512:1024], khT[p][64:128, csl],
                                qhT[p][64:128, gsl],
                                start=True, stop=True,
                            )
                            pt = ptp.tile([128, 1024], BF16, tag="pt")
                            nc.scalar.activation(
                                pt[:], st[:],
                                mybir.ActivationFunctionType.Exp,
                                bias=ebias[:], scale=1.0,
                            )
                            pts.append(pt)
                        dn = dnp.tile([128, 512], F32, tag="dn")
                        u = up.tile([128, 512], F32, tag="u")
                        for c in range(NC):
                            pt = pts[c]
                            first, last = c == 0, c == NC - 1
                            voff = c * SLC + p * 128
                            nc.tensor.matmul(
                                dn[0:64, :], ones[:, :], pt[:, 0:512],
                                start=first, stop=last, skip_group_check=True,
                            )
                            nc.tensor.matmul(
                                dn[64:128, :], ones[:, :], pt[:, 512:1024],
                                start=first, stop=last, skip_group_check=True,
                            )
                            nc.tensor.matmul(
                                u[0:64, :], vS[:, ds(voff, 64)], pt[:, 0:512],
                                start=first, stop=last, skip_group_check=True,
                            )
                            nc.tensor.matmul(
                                u[64:128, :], vS[:, ds(voff + 64, 64)],
                                pt[:, 512:1024],
                                start=first, stop=last, skip_group_check=True,
                            )
                        # normalize on DVE (PE-independent); Wv deferred
                        rcp = sbt.tile([128, 512], F32, tag="rcp", bufs=2)
                        nc.vector.reciprocal(rcp[:], dn[:])
                        un = sbt.tile([128, 512], BF16, tag="un", bufs=13)
                        nc.vector.tensor_tensor(
                            un[:], u[:], rcp[:], op=mybir.AluOpType.mult
                        )
                        uns[(p, g)] = un

                # --- deferred Wv projection + bv for all pairs ---
                attn_ctx.close()
                with tc.tile_pool(name="otp", bufs=2, space="PSUM") as otp:
                    for p in range(NP):
                        for g in range(NG):
                            un = uns[(p, g)]
                            ot = otp.tile([128, 512], F32, tag="ot")
                            nc.tensor.matmul(
                                ot[0:64, :], wv[0:64, :], un[0:64, :],
                                start=True, stop=True,
                            )
                            nc.tensor.matmul(
                                ot[64:128, :], wv[64:128, :], un[64:128, :],
                                start=True, stop=True,
                            )
                            nc.vector.tensor_scalar_add(
                                XT[p][:, ds(g * 512, 512)], ot[:], bvs[:]
                            )

            # --- output projection ---
            with (
                tc.tile_pool(name="pop", bufs=2, space="PSUM") as pop,
                tc.tile_pool(name="outp", bufs=3) as outp,
            ):
                for qb in range(NC):
                    qsl = ds(qb * 128, 128)
                    po = pop.tile([128, D], F32, tag="po")
                    for p in range(NP):
                        first, last = p == 0, p == NP - 1
                        nc.tensor.matmul(
                            po[:, 0:512], XT[p][:, qsl], woS[p][:, 0:512],
                            start=first, stop=last,
                        )
                        nc.tensor.matmul(
                            po[:, 512:768], XT[p][:, qsl], woS[p][:, 512:768],
                            start=first, stop=last,
                        )
                    ou = outp.tile([128, D], F32, tag="ou")
                    nc.vector.tensor_copy(ou[:], po[:])
                    nc.sync.dma_start(
                        out=out_d.rearrange("(n p) o -> n p o", p=128)[qb],
                        in_=ou[:],
                    )

    return nc


def kernel(q, k, v, Wq, bq, Wk, bk, Wv, bv, Wo, bo):
    q = np.asarray(q, dtype=np.float32)
    k = np.asarray(k, dtype=np.float32)
    v = np.asarray(v, dtype=np.float32)
    Wq = np.asarray(Wq, dtype=np.float32)
    bq = np.asarray(bq, dtype=np.float32)
    Wk = np.asarray(Wk, dtype=np.float32)
    bk = np.asarray(bk, dtype=np.float32)
    Wv = np.asarray(Wv, dtype=np.float32)
    bv = np.asarray(bv, dtype=np.float32)
    Wo = np.asarray(Wo, dtype=np.float32)
    bo = np.asarray(bo, dtype=np.float32)

    s = 1.0 / np.sqrt(DK)
    wqT2 = np.concatenate([Wq.T * s, Wq.T * s], axis=0).astype(np.float16)
    wkT2 = np.concatenate([Wk.T, Wk.T], axis=0).astype(np.float16)
    wvT2 = np.concatenate([Wv.T, Wv.T], axis=0).astype(ml_dtypes.bfloat16)
    bq2 = np.concatenate([bq * s, bq * s])[:, None].astype(np.float32)
    bk2 = np.concatenate([bk, bk])[:, None].astype(np.float32)
    bv2 = np.concatenate([bv, bv])[:, None].astype(np.float32)

    in_maps = []
    for c in range(N_CORES):
        b, hg = c // 2, c % 2
        cols = slice(hg * SLC, (hg + 1) * SLC)
        in_maps.append(
            {
                "qT": np.ascontiguousarray(q[b][:, cols].T).astype(np.float16),
                "kT": np.ascontiguousarray(k[b][:, cols].T).astype(np.float16),
                "vN": np.ascontiguousarray(v[b][:, cols]).astype(ml_dtypes.bfloat16),
                "wqT": wqT2,
                "wkT": wkT2,
                "wvT": wvT2,
                "bq2": bq2,
                "bk2": bk2,
                "bv2": bv2,
                "woT": np.ascontiguousarray(Wo[:, cols].T).astype(ml_dtypes.bfloat16),
            }
        )

    nc = _build_bass()
    trace = bool(os.environ.get("KERNEL_TRACE"))
    if trace:
        _install_trace_hook()
    tmpdir = os.environ.get("KERNEL_TRACE_DIR") or None
    res = run_bass_kernel_spmd(
        nc, in_maps, list(range(N_CORES)), trace=trace, tmpdir=tmpdir
    )
    if trace:
        print("KERNEL exec_time_ns:", res.exec_time_ns)
        kernel.last_results = res

    out = np.zeros((B, T, D), dtype=np.float32)
    for b in range(B):
        out[b] = res.results[2 * b]["out"] + res.results[2 * b + 1]["out"] + bo[None, :]
    return out


# revision 15
# speedup vs baseline: 1.0897x; 1.0461x over previous
"""Multi-head self-attention (B=4, T=2048, D=768, H=12, d_k=64) on 8 trn2 cores.

Sharding: core c handles batch c//2 and head-group c%2 (6 heads = 3 pairs).
Each core computes its heads' attention plus its rows of the output
projection; the host sums the two partial projections per batch and adds bo.

Device dataflow (fp16 matmul operands, fp32 PSUM accumulation):
  - host passes q/k transposed (d-major) so no on-chip transposes are needed
  - per-head-pair projections / S^T / PV are packed into the 128x128 PE via
    partition-offset tile placement (two K=64 or M=64 matmuls run concurrently)
  - softmax: exp(s - 5) on ACT (shift keeps fp16 in range; ratios unchanged;
    1/sqrt(d_k) folded into Wq/bq on the host), denominators via all-ones
    stationary matmuls (result lands replicated across partitions),
    normalization deferred to after PV using a fast DVE reciprocal
  - P @ V uses raw v; Wv is applied after PV (linearity), bv added exactly
    because softmax rows sum to 1
"""

import os

import ml_dtypes
import numpy as np

import concourse.bass as bass
import concourse.mybir as mybir
from concourse.bass import ds
from concourse.bass_utils import run_bass_kernel_spmd
from concourse.tile import TileContext
from concourse.vector_clock import ScopedClock, VectorClock

B, T, D = 4, 2048, 768
H, DK = 12, 64
HPC = 6          # heads per core
SLC = HPC * DK   # 384 feature columns per core
N_CORES = 8
ESHIFT = -5.0    # exp(s + ESHIFT): keeps exp/denexisting sums in fp16 range

F16 = mybir.dt.float16
BF16 = mybir.dt.bfloat16
F32 = mybir.dt.float32


def _patch_tile_drain():
    """Walrus CoreV3 CTRL lowering in this build rejects >1 sem wait on the
    TileContext-exit Drain. Split the waits across single-wait nops."""
    if getattr(TileContext, "_drain_patched", False):
        return

    def _drain_and_barrier(self, tick_clock, wait_clock):
        vc = tick_clock.global_clock
        for proc in range(len(vc)):
            t = vc[proc]
            if t > 0:
                nop_inst = self.nc.sync.nop(nofuse=True, hint="drain_wait_split")
                vec = [0] * len(vc)
                vec[proc] = t
                wait_clock.add_sem_waits(
                    nop_inst.ins, ScopedClock({None: VectorClock(vec)})
                )
        self.nc.sync.drain()
        self.nc.all_engine_barrier()
        assert self.sems is not None
        popped = self.nc._tile_sem_poison_stack.pop()
        assert popped is self._sem_poison
        self.nc.clear_and_free_semaphores(list(self.sems.allocated().values()))
        self.nc.all_engine_barrier()

    TileContext._drain_and_barrier = _drain_and_barrier

    # The same walrus build accepts at most ONE sem wait per instruction.
    # Tile's scheduler attaches 2-4. Peel all but the last wait onto
    # single-wait same-engine NoOps at instruction-commit time.
    orig_add = TileContext._add_instruction

    def _add_instruction(self, inst):
        si = getattr(inst, "sync_info", None)
        if si is not None and si.on_wait is not None and len(si.on_wait) > 1:
            waits = list(si.on_wait)
            for w in waits[:-1]:
                nop = mybir.InstNoOp(
                    name=self.nc.get_next_instruction_name(),
                    ins=[],
                    outs=[],
                    text_hint="wait_split",
                    bass_nofuse=True,
                )
                nop.engine = inst.engine
                nop.sync_info = mybir.SyncInfo(on_wait=[w], on_update=[])
                orig_add(self, nop)
            si.on_wait = waits[-1:]
        orig_add(self, inst)

    TileContext._add_instruction = _add_instruction
    TileContext._drain_patched = True


def _install_trace_hook():
    """Provide the antenv.axon_hooks NTFF profile hook this container lacks,
    and skip the bucket artifact upload. Only used when KERNEL_TRACE is set."""
    import sys
    import types

    import concourse.bass_utils as bass_utils

    bass_utils.upload_artifacts = lambda tmpdir: f"local://{tmpdir}"
    if "antenv.axon_hooks" not in sys.modules:
        mod = types.ModuleType("antenv.axon_hooks")
        mod._hook = None
        mod.set_axon_ntff_profile_hook = lambda h: setattr(mod, "_hook", h)
        mod.get_axon_ntff_profile_hook = lambda: mod._hook
        sys.modules["antenv.axon_hooks"] = mod
    from trn_agent_boot.trn_boot import _ntff_profile_via_ctypes

    sys.modules["antenv.axon_hooks"].set_axon_ntff_profile_hook(
        _ntff_profile_via_ctypes("/opt/axon/libaxon_pjrt.so")
    )


def _build_bass():
    _patch_tile_drain()
    nc = bass.Bass("TRN2", target_bir_lowering=False, debug=False)

    qT_d = nc.dram_tensor("qT", [SLC, T], F16, kind="ExternalInput")
    kT_d = nc.dram_tensor("kT", [SLC, T], F16, kind="ExternalInput")
    vN_d = nc.dram_tensor("vN", [T, SLC], BF16, kind="ExternalInput")
    wq_d = nc.dram_tensor("wqT", [128, DK], F16, kind="ExternalInput")
    wk_d = nc.dram_tensor("wkT", [128, DK], F16, kind="ExternalInput")
    wv_d = nc.dram_tensor("wvT", [128, DK], BF16, kind="ExternalInput")
    bq_d = nc.dram_tensor("bq2", [128, 1], F32, kind="ExternalInput")
    bk_d = nc.dram_tensor("bk2", [128, 1], F32, kind="ExternalInput")
    bv_d = nc.dram_tensor("bv2", [128, 1], F32, kind="ExternalInput")
    wo_d = nc.dram_tensor("woT", [SLC, D], BF16, kind="ExternalInput")
    out_d = nc.dram_tensor("out", [T, D], F32, kind="ExternalOutput")

    NP = SLC // 128  # 3 head-pairs
    NG = T // 512    # 4 query groups
    NC = T // 128    # 16 key chunks

    with TileContext(nc) as tc:
        with (
            tc.tile_pool(name="consts", bufs=1) as cst,
            tc.tile_pool(name="inp", bufs=1) as inp,
        ):
            # --- constants / weights ---
            wq = cst.tile([128, DK], F16, tag="wq")
            wk = cst.tile([128, DK], F16, tag="wk")
            wv = cst.tile([128, DK], BF16, tag="wv")
            nc.gpsimd.dma_start(out=wq[:], in_=wq_d[:])
            nc.gpsimd.dma_start(out=wk[:], in_=wk_d[:])
            nc.gpsimd.dma_start(out=wv[:], in_=wv_d[:])
            bqs = cst.tile([128, 1], F32, tag="bqs")
            bks = cst.tile([128, 1], F32, tag="bks")
            bvs = cst.tile([128, 1], F32, tag="bvs")
            nc.gpsimd.dma_start(out=bqs[:], in_=bq_d[:])
            nc.gpsimd.dma_start(out=bks[:], in_=bk_d[:])
            nc.gpsimd.dma_start(out=bvs[:], in_=bv_d[:])
            ones = cst.tile([128, DK], BF16, tag="ones")
            nc.vector.memset(ones[:], 1.0)
            ebias = cst.tile([128, 1], F32, tag="ebias")
            nc.vector.memset(ebias[:], ESHIFT)

            # --- inputs ---
            # q/k loads are interleaved with the per-pair pipeline below; v
            # and Wo are queued right after pair 0's q/k so they land before
            # first use.
            qTr = [
                inp.tile([128, T], F16, tag=f"qTr{p}", name=f"qTr{p}")
                for p in range(NP)
            ]
            kTr = [
                inp.tile([128, T], F16, tag=f"kTr{p}", name=f"kTr{p}")
                for p in range(NP)
            ]
            nc.sync.dma_start(
                out=qTr[0][:], in_=qT_d.rearrange("(n p) t -> n p t", p=128)[0]
            )
            nc.sync.dma_start(
                out=kTr[0][:], in_=kT_d.rearrange("(n p) t -> n p t", p=128)[0]
            )
            vS = inp.tile([128, NC * SLC], BF16, tag="vS")
            nc.sync.dma_start(
                out=vS[:].rearrange("p (n d) -> p n d", n=NC),
                in_=vN_d.rearrange("(n p) d -> p n d", p=128),
            )
            for p in range(1, NP):
                nc.sync.dma_start(
                    out=qTr[p][:], in_=qT_d.rearrange("(n p) t -> n p t", p=128)[p]
                )
                nc.sync.dma_start(
                    out=kTr[p][:], in_=kT_d.rearrange("(n p) t -> n p t", p=128)[p]
                )
            woS = []
            for p in range(NP):
                tw = inp.tile([128, D], BF16, tag=f"woS{p}", name=f"woS{p}")
                nc.sync.dma_start(
                    out=tw[:], in_=wo_d.rearrange("(n p) o -> n p o", p=128)[p]
                )
                woS.append(tw)

            qhT = [
                inp.tile([128, T], F16, tag=f"qhT{p}", name=f"qhT{p}")
                for p in range(NP)
            ]
            khT = [
                inp.tile([128, T], F16, tag=f"khT{p}", name=f"khT{p}")
                for p in range(NP)
            ]
            XT = [
                inp.tile([128, T], BF16, tag=f"XT{p}", name=f"XT{p}")
                for p in range(NP)
            ]

            # --- attention (projections fused per pair; Wv stage deferred so
            # the PE never waits on the DVE reciprocal chain) ---
            with (
                tc.tile_pool(name="ptp", bufs=17) as ptp,
                tc.tile_pool(name="sbt", bufs=13) as sbt,
            ):
                uns = {}
                from contextlib import ExitStack

                attn_ctx = ExitStack()
                stp = attn_ctx.enter_context(
                    tc.tile_pool(name="stp", bufs=2, space="PSUM")
                )
                dnp = attn_ctx.enter_context(
                    tc.tile_pool(name="dnp", bufs=2, space="PSUM")
                )
                up = attn_ctx.enter_context(
                    tc.tile_pool(name="up", bufs=2, space="PSUM")
                )
                def emit_proj(p):
                    # q/k projections for pair p (pair-packed (0,0)/(64,64));
                    # psum comes from the "st" slots
                    for src, w, bias, dst in (
                        (qTr[p], wq, bqs, qhT[p]),
                        (kTr[p], wk, bks, khT[p]),
                    ):
                        for h in range(2):
                            pjt = stp.tile([128, 1024], F32, tag="st", name="pj")
                            for n in range(2):
                                sl = ds(h * 1024 + n * 512, 512)
                                psl = ds(n * 512, 512)
                                nc.tensor.matmul(
                                    pjt[0:64, psl], w[0:64, :], src[0:64, sl],
                                    start=True, stop=True,
                                )
                                nc.tensor.matmul(
                                    pjt[64:128, psl], w[64:128, :], src[64:128, sl],
                                    start=True, stop=True,
                                )
                            nc.vector.tensor_scalar_add(
                                dst[:, ds(h * 1024, 1024)], pjt[:], bias[:]
                            )

                emit_proj(0)
                for p in range(NP):
                    for g in range(NG):
                        gsl = ds(g * 512, 512)
                        pts = []
                        for c in range(NC):
                            csl = ds(c * 128, 128)
                            st = stp.tile([128, 1024], F32, tag="st")
                            nc.tensor.matmul(
                                st[:, 0:512], khT[p][0:64, csl], qhT[p][0:64, gsl],
                                start=True, stop=True,
                            )
                            nc.tensor.matmul(
                                st[:, 512:1024], khT[p][64:128, csl],
                                qhT[p][64:128, gsl],
                                start=True, stop=True,
                            )
                            pt = ptp.tile([128, 1024], BF16, tag="pt")
                            nc.scalar.activation(
                                pt[:], st[:],
                                mybir.ActivationFunctionType.Exp,
                                bias=ebias[:], scale=1.0,
                            )
                            pts.append(pt)
                        dn = dnp.tile([128, 512], F32, tag="dn")
                        u = up.tile([128, 512], F32, tag="u")
                        for c in range(NC):
                            pt = pts[c]
                            first, last = c == 0, c == NC - 1
                            voff = c * SLC + p * 128
                            nc.tensor.matmul(
                                dn[0:64, :], ones[:, :], pt[:, 0:512],
                                start=first, stop=last, skip_group_check=True,
                            )
                            nc.tensor.matmul(
                                dn[64:128, :], ones[:, :], pt[:, 512:1024],
                                start=first, stop=last, skip_group_check=True,
                            )
                            nc.tensor.matmul(
                                u[0:64, :], vS[:, ds(voff, 64)], pt[:, 0:512],
                                start=first, stop=last, skip_group_check=True,
                            )
                            nc.tensor.matmul(
                                u[64:128, :], vS[:, ds(voff + 64, 64)],
                                pt[:, 512:1024],
                                start=first, stop=last, skip_group_check=True,
                            )
                        # normalize on DVE (PE-independent); Wv deferred
                        rcp = sbt.tile([128, 512], F32, tag="rcp", bufs=2)
                        nc.vector.reciprocal(rcp[:], dn[:])
                        un = sbt.tile([128, 512], BF16, tag="un", bufs=13)
                        nc.vector.tensor_tensor(
                            un[:], u[:], rcp[:], op=mybir.AluOpType.mult
                        )
                        uns[(p, g)] = un
                        if g == 1 and p + 1 < NP:
                            emit_proj(p + 1)

                # --- deferred Wv projection + bv for all pairs ---
                attn_ctx.close()
                with tc.tile_pool(name="otp", bufs=2, space="PSUM") as otp:
                    for p in range(NP):
                        for g in range(NG):
                            un = uns[(p, g)]
                            ot = otp.tile([128, 512], F32, tag="ot")
                            nc.tensor.matmul(
                                ot[0:64, :], wv[0:64, :], un[0:64, :],
                                start=True, stop=True,
                            )
                            nc.tensor.matmul(
                                ot[64:128, :], wv[64:128, :], un[64:128, :],
                                start=True, stop=True,
                            )
                            nc.vector.tensor_scalar_add(
                                XT[p][:, ds(g * 512, 512)], ot[:], bvs[:]
                            )

            # --- output projection ---
            with (
                tc.tile_pool(name="pop", bufs=2, space="PSUM") as pop,
                tc.tile_pool(name="outp", bufs=3) as outp,
            ):
                for qb in range(NC):
                    qsl = ds(qb * 128, 128)
                    po = pop.tile([128, D], F32, tag="po")
                    for p in range(NP):
                        first, last = p == 0, p == NP - 1
                        nc.tensor.matmul(
                            po[:, 0:512], XT[p][:, qsl], woS[p][:, 0:512],
                            start=first, stop=last,
                        )
                        nc.tensor.matmul(
                            po[:, 512:768], XT[p][:, qsl], woS[p][:, 512:768],
                            start=first, stop=last,
                        )
                    ou = outp.tile([128, D], F32, tag="ou")
                    nc.vector.tensor_copy(ou[:], po[:])
                    nc.sync.dma_start(
                        out=out_d.rearrange("(n p) o -> n p o", p=128)[qb],
                        in_=ou[:],
                    )

    return nc


def kernel(q, k, v, Wq, bq, Wk, bk, Wv, bv, Wo, bo):
    q = np.asarray(q, dtype=np.float32)
    k = np.asarray(k, dtype=np.float32)
    v = np.asarray(v, dtype=np.float32)
    Wq = np.asarray(Wq, dtype=np.float32)
    bq = np.asarray(bq, dtype=np.float32)
    Wk = np.asarray(Wk, dtype=np.float32)
    bk = np.asarray(bk, dtype=np.float32)
    Wv = np.asarray(Wv, dtype=np.float32)
    bv = np.asarray(bv, dtype=np.float32)
    Wo = np.asarray(Wo, dtype=np.float32)
    bo = np.asarray(bo, dtype=np.float32)

    s = 1.0 / np.sqrt(DK)
    wqT2 = np.concatenate([Wq.T * s, Wq.T * s], axis=0).astype(np.float16)
    wkT2 = np.concatenate([Wk.T, Wk.T], axis=0).astype(np.float16)
    wvT2 = np.concatenate([Wv.T, Wv.T], axis=0).astype(ml_dtypes.bfloat16)
    bq2 = np.concatenate([bq * s, bq * s])[:, None].astype(np.float32)
    bk2 = np.concatenate([bk, bk])[:, None].astype(np.float32)
    bv2 = np.concatenate([bv, bv])[:, None].astype(np.float32)

    in_maps = []
    for c in range(N_CORES):
        b, hg = c // 2, c % 2
        cols = slice(hg * SLC, (hg + 1) * SLC)
        in_maps.append(
            {
                "qT": np.ascontiguousarray(q[b][:, cols].T).astype(np.float16),
                "kT": np.ascontiguousarray(k[b][:, cols].T).astype(np.float16),
                "vN": np.ascontiguousarray(v[b][:, cols]).astype(ml_dtypes.bfloat16),
                "wqT": wqT2,
                "wkT": wkT2,
                "wvT": wvT2,
                "bq2": bq2,
                "bk2": bk2,
                "bv2": bv2,
                "woT": np.ascontiguousarray(Wo[:, cols].T).astype(ml_dtypes.bfloat16),
            }
        )

    nc = _build_bass()
    trace = bool(os.environ.get("KERNEL_TRACE"))
    if trace:
        _install_trace_hook()
    tmpdir = os.environ.get("KERNEL_TRACE_DIR") or None
    res = run_bass_kernel_spmd(
        nc, in_maps, list(range(N_CORES)), trace=trace, tmpdir=tmpdir
    )
    if trace:
        print("KERNEL exec_time_ns:", res.exec_time_ns)
        kernel.last_results = res

    out = np.zeros((B, T, D), dtype=np.float32)
    for b in range(B):
        out[b] = res.results[2 * b]["out"] + res.results[2 * b + 1]["out"] + bo[None, :]
    return out


# revision 16
# speedup vs baseline: 1.1099x; 1.0185x over previous
"""Multi-head self-attention (B=4, T=2048, D=768, H=12, d_k=64) on 8 trn2 cores.

Sharding: core c handles batch c//2 and head-group c%2 (6 heads = 3 pairs).
Each core computes its heads' attention plus its rows of the output
projection; the host sums the two partial projections per batch and adds bo.

Device dataflow (fp16 matmul operands, fp32 PSUM accumulation):
  - host passes q/k transposed (d-major) so no on-chip transposes are needed
  - per-head-pair projections / S^T / PV are packed into the 128x128 PE via
    partition-offset tile placement (two K=64 or M=64 matmuls run concurrently)
  - softmax: exp(s - 5) on ACT (shift keeps fp16 in range; ratios unchanged;
    1/sqrt(d_k) folded into Wq/bq on the host), denominators via all-ones
    stationary matmuls (result lands replicated across partitions),
    normalization deferred to after PV using a fast DVE reciprocal
  - P @ V uses raw v; Wv is applied after PV (linearity), bv added exactly
    because softmax rows sum to 1
"""

import os

import ml_dtypes
import numpy as np

import concourse.bass as bass
import concourse.mybir as mybir
from concourse.bass import ds
from concourse.bass_utils import run_bass_kernel_spmd
from concourse.tile import TileContext
from concourse.vector_clock import ScopedClock, VectorClock

B, T, D = 4, 2048, 768
H, DK = 12, 64
HPC = 6          # heads per core
SLC = HPC * DK   # 384 feature columns per core
N_CORES = 8
ESHIFT = -5.0    # exp(s + ESHIFT): keeps exp/denexisting sums in fp16 range

F16 = mybir.dt.float16
BF16 = mybir.dt.bfloat16
F32 = mybir.dt.float32


def _patch_tile_drain():
    """Walrus CoreV3 CTRL lowering in this build rejects >1 sem wait on the
    TileContext-exit Drain. Split the waits across single-wait nops."""
    if getattr(TileContext, "_drain_patched", False):
        return

    def _drain_and_barrier(self, tick_clock, wait_clock):
        vc = tick_clock.global_clock
        for proc in range(len(vc)):
            t = vc[proc]
            if t > 0:
                nop_inst = self.nc.sync.nop(nofuse=True, hint="drain_wait_split")
                vec = [0] * len(vc)
                vec[proc] = t
                wait_clock.add_sem_waits(
                    nop_inst.ins, ScopedClock({None: VectorClock(vec)})
                )
        self.nc.sync.drain()
        self.nc.all_engine_barrier()
        assert self.sems is not None
        popped = self.nc._tile_sem_poison_stack.pop()
        assert popped is self._sem_poison
        self.nc.clear_and_free_semaphores(list(self.sems.allocated().values()))
        self.nc.all_engine_barrier()

    TileContext._drain_and_barrier = _drain_and_barrier

    # The same walrus build accepts at most ONE sem wait per instruction.
    # Tile's scheduler attaches 2-4. Peel all but the last wait onto
    # single-wait same-engine NoOps at instruction-commit time.
    orig_add = TileContext._add_instruction

    def _add_instruction(self, inst):
        si = getattr(inst, "sync_info", None)
        if si is not None and si.on_wait is not None and len(si.on_wait) > 1:
            waits = list(si.on_wait)
            for w in waits[:-1]:
                nop = mybir.InstNoOp(
                    name=self.nc.get_next_instruction_name(),
                    ins=[],
                    outs=[],
                    text_hint="wait_split",
                    bass_nofuse=True,
                )
                nop.engine = inst.engine
                nop.sync_info = mybir.SyncInfo(on_wait=[w], on_update=[])
                orig_add(self, nop)
            si.on_wait = waits[-1:]
        orig_add(self, inst)

    TileContext._add_instruction = _add_instruction
    TileContext._drain_patched = True


def _install_trace_hook():
    """Provide the antenv.axon_hooks NTFF profile hook this container lacks,
    and skip the bucket artifact upload. Only used when KERNEL_TRACE is set."""
    import sys
    import types

    import concourse.bass_utils as bass_utils

    bass_utils.upload_artifacts = lambda tmpdir: f"local://{tmpdir}"
    if "antenv.axon_hooks" not in sys.modules:
        mod = types.ModuleType("antenv.axon_hooks")
        mod._hook = None
        mod.set_axon_ntff_profile_hook = lambda h: setattr(mod, "_hook", h)
        mod.get_axon_ntff_profile_hook = lambda: mod._hook
        sys.modules["antenv.axon_hooks"] = mod
    from trn_agent_boot.trn_boot import _ntff_profile_via_ctypes

    sys.modules["antenv.axon_hooks"].set_axon_ntff_profile_hook(
        _ntff_profile_via_ctypes("/opt/axon/libaxon_pjrt.so")
    )


def _build_bass():
    _patch_tile_drain()
    nc = bass.Bass("TRN2", target_bir_lowering=False, debug=False)

    qT_d = nc.dram_tensor("qT", [SLC, T], F16, kind="ExternalInput")
    kT_d = nc.dram_tensor("kT", [SLC, T], F16, kind="ExternalInput")
    vN_d = nc.dram_tensor("vN", [T, SLC], BF16, kind="ExternalInput")
    wq_d = nc.dram_tensor("wqT", [128, DK], F16, kind="ExternalInput")
    wk_d = nc.dram_tensor("wkT", [128, DK], F16, kind="ExternalInput")
    wv_d = nc.dram_tensor("wvT", [128, DK], BF16, kind="ExternalInput")
    bq_d = nc.dram_tensor("bq2", [128, 1], F32, kind="ExternalInput")
    bk_d = nc.dram_tensor("bk2", [128, 1], F32, kind="ExternalInput")
    bv_d = nc.dram_tensor("bv2", [128, 1], F32, kind="ExternalInput")
    wo_d = nc.dram_tensor("woT", [SLC, D], BF16, kind="ExternalInput")
    out_d = nc.dram_tensor("out", [T, D], F32, kind="ExternalOutput")

    NP = SLC // 128  # 3 head-pairs
    NG = T // 512    # 4 query groups
    NC = T // 128    # 16 key chunks

    with TileContext(nc) as tc:
        with (
            tc.tile_pool(name="consts", bufs=1) as cst,
            tc.tile_pool(name="inp", bufs=1) as inp,
        ):
            # --- constants / weights ---
            wq = cst.tile([128, DK], F16, tag="wq")
            wk = cst.tile([128, DK], F16, tag="wk")
            wv = cst.tile([128, DK], BF16, tag="wv")
            nc.gpsimd.dma_start(out=wq[:], in_=wq_d[:])
            nc.gpsimd.dma_start(out=wk[:], in_=wk_d[:])
            nc.gpsimd.dma_start(out=wv[:], in_=wv_d[:])
            bqs = cst.tile([128, 1], F32, tag="bqs")
            bks = cst.tile([128, 1], F32, tag="bks")
            bvs = cst.tile([128, 1], F32, tag="bvs")
            nc.gpsimd.dma_start(out=bqs[:], in_=bq_d[:])
            nc.gpsimd.dma_start(out=bks[:], in_=bk_d[:])
            nc.gpsimd.dma_start(out=bvs[:], in_=bv_d[:])
            ones = cst.tile([128, DK], BF16, tag="ones")
            nc.vector.memset(ones[:], 1.0)
            ebias = cst.tile([128, 1], F32, tag="ebias")
            nc.vector.memset(ebias[:], ESHIFT)

            # --- inputs ---
            # q/k loads are interleaved with the per-pair pipeline below; v
            # and Wo are queued right after pair 0's q/k so they land before
            # first use.
            qTr = [
                inp.tile([128, T], F16, tag=f"qTr{p}", name=f"qTr{p}")
                for p in range(NP)
            ]
            kTr = [
                inp.tile([128, T], F16, tag=f"kTr{p}", name=f"kTr{p}")
                for p in range(NP)
            ]
            nc.sync.dma_start(
                out=qTr[0][:], in_=qT_d.rearrange("(n p) t -> n p t", p=128)[0]
            )
            nc.sync.dma_start(
                out=kTr[0][:], in_=kT_d.rearrange("(n p) t -> n p t", p=128)[0]
            )
            vS = inp.tile([128, NC * SLC], BF16, tag="vS")
            nc.sync.dma_start(
                out=vS[:].rearrange("p (n d) -> p n d", n=NC),
                in_=vN_d.rearrange("(n p) d -> p n d", p=128),
            )
            for p in range(1, NP):
                nc.sync.dma_start(
                    out=qTr[p][:], in_=qT_d.rearrange("(n p) t -> n p t", p=128)[p]
                )
                nc.sync.dma_start(
                    out=kTr[p][:], in_=kT_d.rearrange("(n p) t -> n p t", p=128)[p]
                )
            woS = []
            for p in range(NP):
                tw = inp.tile([128, D], BF16, tag=f"woS{p}", name=f"woS{p}")
                nc.sync.dma_start(
                    out=tw[:], in_=wo_d.rearrange("(n p) o -> n p o", p=128)[p]
                )
                woS.append(tw)

            qhT = [
                inp.tile([128, T], F16, tag=f"qhT{p}", name=f"qhT{p}")
                for p in range(NP)
            ]
            khT = [
                inp.tile([128, T], F16, tag=f"khT{p}", name=f"khT{p}")
                for p in range(NP)
            ]
            XT = [
                inp.tile([128, T], BF16, tag=f"XT{p}", name=f"XT{p}")
                for p in range(NP)
            ]

            # --- attention ---
            # PSUM budget (8 banks): st 2x[128,1024]=4, dn [128,512]=1,
            # u [128,512]=1, aux 2x[128,512]=2 (projection chunks + Wv).
            # dn/u are copied to SBUF right after their accumulation so the
            # single-buffered banks free quickly; the reciprocal chain runs
            # entirely in SBUF off the PE's critical path. The Wv stage for
            # group g is emitted one group later (lag-1) so its TT input is
            # always ready when the PE reaches it.
            with (
                tc.tile_pool(name="ptp", bufs=17) as ptp,
                tc.tile_pool(name="sbt", bufs=13) as sbt,
                tc.tile_pool(name="stp", bufs=2, space="PSUM") as stp,
                tc.tile_pool(name="dnp", bufs=1, space="PSUM") as dnp,
                tc.tile_pool(name="up", bufs=1, space="PSUM") as up,
                tc.tile_pool(name="aux", bufs=2, space="PSUM") as aux,
            ):
                uns = {}
                pending_wv = []

                def emit_proj(p):
                    # q/k projections for pair p (pair-packed (0,0)/(64,64))
                    for src, w, bias, dst in (
                        (qTr[p], wq, bqs, qhT[p]),
                        (kTr[p], wk, bks, khT[p]),
                    ):
                        for n in range(NG):
                            pjt = aux.tile([128, 512], F32, tag="aux", name="pj")
                            sl = ds(n * 512, 512)
                            nc.tensor.matmul(
                                pjt[0:64, :], w[0:64, :], src[0:64, sl],
                                start=True, stop=True,
                            )
                            nc.tensor.matmul(
                                pjt[64:128, :], w[64:128, :], src[64:128, sl],
                                start=True, stop=True,
                            )
                            nc.vector.tensor_scalar_add(dst[:, sl], pjt[:], bias[:])

                def emit_wv(p, g):
                    un = uns[(p, g)]
                    ot = aux.tile([128, 512], F32, tag="aux", name="ot")
                    nc.tensor.matmul(
                        ot[0:64, :], wv[0:64, :], un[0:64, :],
                        start=True, stop=True,
                    )
                    nc.tensor.matmul(
                        ot[64:128, :], wv[64:128, :], un[64:128, :],
                        start=True, stop=True,
                    )
                    nc.vector.tensor_scalar_add(
                        XT[p][:, ds(g * 512, 512)], ot[:], bvs[:]
                    )

                emit_proj(0)
                for p in range(NP):
                    for g in range(NG):
                        gsl = ds(g * 512, 512)
                        pts = []
                        for c in range(NC):
                            csl = ds(c * 128, 128)
                            st = stp.tile([128, 1024], F32, tag="st")
                            nc.tensor.matmul(
                                st[:, 0:512], khT[p][0:64, csl], qhT[p][0:64, gsl],
                                start=True, stop=True,
                            )
                            nc.tensor.matmul(
                                st[:, 512:1024], khT[p][64:128, csl],
                                qhT[p][64:128, gsl],
                                start=True, stop=True,
                            )
                            pt = ptp.tile([128, 1024], BF16, tag="pt")
                            nc.scalar.activation(
                                pt[:], st[:],
                                mybir.ActivationFunctionType.Exp,
                                bias=ebias[:], scale=1.0,
                            )
                            pts.append(pt)
                        dn = dnp.tile([128, 512], F32, tag="dn")
                        u = up.tile([128, 512], F32, tag="u")
                        for c in range(NC):
                            pt = pts[c]
                            first, last = c == 0, c == NC - 1
                            voff = c * SLC + p * 128
                            nc.tensor.matmul(
                                dn[0:64, :], ones[:, :], pt[:, 0:512],
                                start=first, stop=last, skip_group_check=True,
                            )
                            nc.tensor.matmul(
                                dn[64:128, :], ones[:, :], pt[:, 512:1024],
                                start=first, stop=last, skip_group_check=True,
                            )
                            nc.tensor.matmul(
                                u[0:64, :], vS[:, ds(voff, 64)], pt[:, 0:512],
                                start=first, stop=last, skip_group_check=True,
                            )
                            nc.tensor.matmul(
                                u[64:128, :], vS[:, ds(voff + 64, 64)],
                                pt[:, 512:1024],
                                start=first, stop=last, skip_group_check=True,
                            )
                        # free the accumulators fast, normalize in SBUF
                        dnS = sbt.tile([128, 512], F32, tag="dnS", bufs=2)
                        nc.vector.tensor_copy(dnS[:], dn[:])
                        uS = sbt.tile([128, 512], F32, tag="uS", bufs=2)
                        nc.vector.tensor_copy(uS[:], u[:])
                        rcp = sbt.tile([128, 512], F32, tag="rcp", bufs=2)
                        nc.vector.reciprocal(rcp[:], dnS[:])
                        un = sbt.tile([128, 512], BF16, tag="un", bufs=4)
                        nc.vector.tensor_tensor(
                            un[:], uS[:], rcp[:], op=mybir.AluOpType.mult
                        )
                        uns[(p, g)] = un
                        pending_wv.append((p, g))
                        if len(pending_wv) > 1:
                            emit_wv(*pending_wv.pop(0))
                        if g == 2 and p + 1 < NP:
                            emit_proj(p + 1)
                for pg in pending_wv:
                    emit_wv(*pg)

            # --- output projection ---
            with (
                tc.tile_pool(name="pop", bufs=2, space="PSUM") as pop,
                tc.tile_pool(name="outp", bufs=3) as outp,
            ):
                for qb in range(NC):
                    qsl = ds(qb * 128, 128)
                    po = pop.tile([128, D], F32, tag="po")
                    for p in range(NP):
                        first, last = p == 0, p == NP - 1
                        nc.tensor.matmul(
                            po[:, 0:512], XT[p][:, qsl], woS[p][:, 0:512],
                            start=first, stop=last,
                        )
                        nc.tensor.matmul(
                            po[:, 512:768], XT[p][:, qsl], woS[p][:, 512:768],
                            start=first, stop=last,
                        )
                    ou = outp.tile([128, D], F32, tag="ou")
                    nc.vector.tensor_copy(ou[:], po[:])
                    nc.sync.dma_start(
                        out=out_d.rearrange("(n p) o -> n p o", p=128)[qb],
                        in_=ou[:],
                    )

    return nc


def kernel(q, k, v, Wq, bq, Wk, bk, Wv, bv, Wo, bo):
    q = np.asarray(q, dtype=np.float32)
    k = np.asarray(k, dtype=np.float32)
    v = np.asarray(v, dtype=np.float32)
    Wq = np.asarray(Wq, dtype=np.float32)
    bq = np.asarray(bq, dtype=np.float32)
    Wk = np.asarray(Wk, dtype=np.float32)
    bk = np.asarray(bk, dtype=np.float32)
    Wv = np.asarray(Wv, dtype=np.float32)
    bv = np.asarray(bv, dtype=np.float32)
    Wo = np.asarray(Wo, dtype=np.float32)
    bo = np.asarray(bo, dtype=np.float32)

    s = 1.0 / np.sqrt(DK)
    wqT2 = np.concatenate([Wq.T * s, Wq.T * s], axis=0).astype(np.float16)
    wkT2 = np.concatenate([Wk.T, Wk.T], axis=0).astype(np.float16)
    wvT2 = np.concatenate([Wv.T, Wv.T], axis=0).astype(ml_dtypes.bfloat16)
    bq2 = np.concatenate([bq * s, bq * s])[:, None].astype(np.float32)
    bk2 = np.concatenate([bk, bk])[:, None].astype(np.float32)
    bv2 = np.concatenate([bv, bv])[:, None].astype(np.float32)

    in_maps = []
    for c in range(N_CORES):
        b, hg = c // 2, c % 2
        cols = slice(hg * SLC, (hg + 1) * SLC)
        in_maps.append(
            {
                "qT": np.ascontiguousarray(q[b][:, cols].T).astype(np.float16),
                "kT": np.ascontiguousarray(k[b][:, cols].T).astype(np.float16),
                "vN": np.ascontiguousarray(v[b][:, cols]).astype(ml_dtypes.bfloat16),
                "wqT": wqT2,
                "wkT": wkT2,
                "wvT": wvT2,
                "bq2": bq2,
                "bk2": bk2,
                "bv2": bv2,
                "woT": np.ascontiguousarray(Wo[:, cols].T).astype(ml_dtypes.bfloat16),
            }
        )

    nc = _build_bass()
    trace = bool(os.environ.get("KERNEL_TRACE"))
    if trace:
        _install_trace_hook()
    tmpdir = os.environ.get("KERNEL_TRACE_DIR") or None
    res = run_bass_kernel_spmd(
        nc, in_maps, list(range(N_CORES)), trace=trace, tmpdir=tmpdir
    )
    if trace:
        print("KERNEL exec_time_ns:", res.exec_time_ns)
        kernel.last_results = res

    out = np.zeros((B, T, D), dtype=np.float32)
    for b in range(B):
        out[b] = res.results[2 * b]["out"] + res.results[2 * b + 1]["out"] + bo[None, :]
    return out
